# revision 19
# baseline (speedup 1.0000x reference)
"""Causal varlen self-attention (qk-norm + rotary + head gating) on 8 trn2 cores.

Sharding: data-parallel by sequence — 8 packed equal-length sequences, one per
NeuronCore; weights replicated. No collectives.

v2: all matmuls in bf16 (1 cycle/row on the PE vs ~2.3 for fp32-HIGH mode),
elementwise work in bf16 (DVE 2x mode), V projected directly into natural
[token, head*D] layout (no PE transposes / PSUM evacuation chain), k-side
rms scale folded into the Exp activation's per-partition scale operand,
reciprocal_approx_fast instead of multi-pass InstReciprocal, and PSUM
evacuation moved to the otherwise-idle GpSimd engine.

Per-core dataflow (S=1024 tokens, C=1024 hidden, H=16 heads, D=64):
  phase 1: qkv^T = W-tiles x x^T (bf16). rotary applied on DVE in bf16;
           per-token sumsq via ones-block matmul of square(pq) (rotation
           preserves per-token norms, so sumsq is taken pre-rotary).
           q gets 1/sqrt(ms/64+eps) applied via broadcast-DMA + DVE mul;
           k's 1/sqrt(ms+64 eps) (D^-.5 folded) is transposed into [kpos, h]
           columns and later fed to Exp as its per-partition scale.
           v computed in natural [tok, H*D] layout: x^T-tile-stationary x
           Wv^T-moving, PSUM evacuated straight into v_aug (ones column
           appended so the softmax denominator falls out of the PV matmul).
  phase 2: per (head, k-tile): scores = k-tile-stationary x q-moving,
           et = Exp(scores * rk) on ACT, causal mask on the diagonal tile,
           PV accumulates [65, S]. GpSimd evacuates attention rows to aos
           and denominator rows to staging; gate*1/denominator applied as
           one broadcast multiply.
  phase 3: out^T = Wo^T-tiles-stationary x aos-moving.
"""

import os
import sys
from contextlib import ExitStack

sys.path.insert(0, "/opt/trn_rl_repo")

K_GPS = os.environ.get("K_GPS", "1") == "1"  # gpsimd PSUM evacuation
K_EXPSC = os.environ.get("K_EXPSC", "1") == "1"  # exp scale-AP rk fold
K_V3D = os.environ.get("K_V3D", "1") == "1"  # 3D strided ACT out for v evac
K_TRS = os.environ.get("K_TRS", "1") == "1"  # 16-partition transpose for rkT
# reciprocal_approx_fast (custom DVE) fails this walrus build's codegen
# ("ISA wrong length") — default to the stock multi-pass InstReciprocal.
K_RECF = os.environ.get("K_RECF", "0") == "1"

import numpy as np
import ml_dtypes
import bass_rust
import concourse.bass as bass
import concourse.tile as tile
from concourse import mybir
from concourse import bass_utils

P = 128
S = 1024  # tokens per sequence (= per core)
C = 1024  # hidden
H = 16
D = 64
NCORES = 8
F32 = mybir.dt.float32
BF16 = mybir.dt.bfloat16
AF = mybir.ActivationFunctionType
BF16NP = ml_dtypes.bfloat16


class TC(tile.TileContext):
    """TileContext that rewrites every instruction to carry at most ONE sem wait.

    This container's walrus rejects instructions with more than one sync wait
    command (matmul LDW structs, CTRL drains, ...). Tile's wait-assignment
    pass attaches one wait per producer proc, so fan-in instructions get
    several. After scheduling, hoist all but the last wait of each
    instruction onto same-engine NOPs inserted immediately before it —
    identical synchronization semantics, one wait per encoded instruction.
    """

    _split_seq = 0
    split_waits = True

    def schedule_and_allocate(self, *args, **kwargs):
        ret = super().schedule_and_allocate(*args, **kwargs)
        if not self.split_waits:
            return ret
        nc = self.nc
        for fn in nc.m.functions:
            for blk in fn.blocks:
                insts = blk.instructions
                out = []
                changed = False
                for ins in insts:
                    si = getattr(ins, "sync_info", None)
                    waits = list(si.on_wait) if si is not None else []
                    if len(waits) > 1:
                        changed = True
                        for w in waits[:-1]:
                            TC._split_seq += 1
                            nop = bass_rust.InstNoOp(
                                name=f"I-splitw-{TC._split_seq}",
                                engine=ins.engine,
                                ins=[],
                                outs=[],
                            )
                            nop.sync_info = bass_rust.SyncInfo(
                                on_wait=[w], on_update=[]
                            )
                            out.append(nop)
                        ins.sync_info = bass_rust.SyncInfo(
                            on_wait=[waits[-1]], on_update=list(si.on_update)
                        )
                    out.append(ins)
                if changed:
                    blk.instructions = out
        return ret


def build_program(split_waits=True):
    nc = bass.Bass("TRN2", target_bir_lowering=False, debug=False)
    dt = nc.dram_tensor
    xt_d = dt("xt", [C, S], BF16, kind="ExternalInput").ap()
    wqk_d = dt("wqk", [16, P, 8, P], BF16, kind="ExternalInput").ap()
    wvt_d = dt("wvt", [8, P, C], BF16, kind="ExternalInput").ap()
    wo_d = dt("wo", [8, P, 8, P], BF16, kind="ExternalInput").ap()
    gw_d = dt("gw", [P, P], BF16, kind="ExternalInput").ap()
    gb_d = dt("gb", [H, 1], F32, kind="ExternalInput").ap()
    cosf_d = dt("cosf", [P, S], BF16, kind="ExternalInput").ap()
    sinp_d = dt("sinp", [P, S], BF16, kind="ExternalInput").ap()
    maskt_d = dt("maskt", [P, P], BF16, kind="ExternalInput").ap()
    bones_d = dt("bones", [P, 2], BF16, kind="ExternalInput").ap()
    identq_d = dt("identq", [32, 32], F32, kind="ExternalInput").ap()
    outt_d = dt("outt", [C, S], F32, kind="ExternalOutput").ap()
    rq_scr = dt("rq_scr", [H, S], BF16).ap()
    rk_scr = dt("rk_scr", [H, S], BF16).ap()
    sc_scr = dt("sc_scr", [H, S], BF16).ap()

    with TC(nc) as tc:
        tc.split_waits = split_waits
        with (
            tc.tile_pool(name="const", bufs=1) as constp,
            tc.tile_pool(name="resid", bufs=1) as resid,
            tc.tile_pool(name="stats", bufs=1) as stats,
        ):
            cosf = constp.tile([P, S], BF16, tag="cosf")
            sinp = constp.tile([P, S], BF16, tag="sinp")
            maskt = constp.tile([P, P], BF16, tag="maskt")
            bones = constp.tile([P, 2], BF16, tag="bones")
            identq = constp.tile([32, 32], F32, tag="identq")
            gw_sb = constp.tile([P, P], BF16, tag="gw")
            gb_sb = constp.tile([H, 1], F32, tag="gb")
            nc.sync.dma_start(cosf[:], cosf_d[:])
            nc.sync.dma_start(sinp[:], sinp_d[:])
            nc.sync.dma_start(maskt[:], maskt_d[:])
            nc.sync.dma_start(bones[:], bones_d[:])
            nc.sync.dma_start(identq[:], identq_d[:])
            nc.sync.dma_start(gw_sb[:], gw_d[:])
            nc.sync.dma_start(gb_sb[:], gb_d[:])

            qr = resid.tile([P, 8, S], BF16, tag="qr")
            kr = resid.tile([P, 8, S], BF16, tag="kr")
            vaug = resid.tile([P, 8, H * 65], BF16, tag="vaug")
            aos = resid.tile([P, 8, S], BF16, tag="aos")

            gate_sb = stats.tile([H, S], F32, tag="gate")
            srtq = stats.tile([H, S], F32, tag="srtq")
            srtk = stats.tile([H, S], F32, tag="srtk")
            rinvq = stats.tile([H, S], F32, tag="rinvq")
            rinvk = stats.tile([H, S], F32, tag="rinvk")
            rq16 = stats.tile([H, S], BF16, tag="rq16")
            rk16 = stats.tile([H, S], BF16, tag="rk16")
            rkT = stats.tile([P, 8, H], F32, tag="rkT")
            sums = stats.tile([H, S], F32, tag="sums")
            lngate = stats.tile([H, S], F32, tag="lngate")
            rd = stats.tile([H, S], F32, tag="rd")
            sc16 = stats.tile([H, S], BF16, tag="sc16")
            eps2q = stats.tile([2, 1], F32, tag="eps2q")
            eps2k = stats.tile([2, 1], F32, tag="eps2k")
            nc.vector.memset(eps2q[:], 1e-6)
            nc.vector.memset(eps2k[:], 6.4e-5)

            # ones columns of v_aug (col 64 of each head's 65-wide block)
            for kt in range(8):
                ones_ap = vaug[:, kt, :].rearrange("p (h e) -> p h e", h=H)[
                    :, :, 64:65
                ]
                nc.vector.memset(ones_ap, 1.0)

            # ---------------- phase 1: projections ----------------
            with (
                tc.tile_pool(name="xp", bufs=1) as xp,
                tc.tile_pool(name="wvp", bufs=1) as wvp,
                tc.tile_pool(name="wqks", bufs=2) as wqks,
                tc.tile_pool(name="pqc", bufs=2) as pqcp,
                tc.tile_pool(name="sqp", bufs=2) as sqp,
                tc.tile_pool(name="tmp", bufs=2) as tmpp,
                tc.tile_pool(name="s2p", bufs=2) as s2p,
                tc.tile_pool(name="bcp", bufs=2) as bcp,
            ):
                xT = xp.tile([P, 8, S], BF16, tag="xT")
                for c in range(8):
                    for ch in range(2):
                        sl = slice(ch * 512, (ch + 1) * 512)
                        nc.sync.dma_start(
                            xT[:, c, sl], xt_d[c * P : (c + 1) * P, sl]
                        )
                wvT = wvp.tile([P, 8, C], BF16, tag="wvT")
                for c in range(8):
                    nc.sync.dma_start(wvT[:, c, :], wvt_d[c])

                # gate logits, one 512-chunk at a time (Sigmoid table first)
                with tc.tile_pool(name="pgate", bufs=2, space="PSUM") as pgatep:
                    for ch in range(2):
                        sl = slice(ch * 512, (ch + 1) * 512)
                        pgate = pgatep.tile([H, 512], F32, tag="pgate")
                        for c in range(8):
                            nc.tensor.matmul(
                                pgate[:],
                                gw_sb[:, c * H : (c + 1) * H],
                                xT[:, c, sl],
                                start=(c == 0),
                                stop=(c == 7),
                            )
                        nc.scalar.activation(
                            gate_sb[:, sl], pgate[:], AF.Sigmoid, bias=gb_sb[:, 0:1]
                        )

                phase1_stack = ExitStack()
                pqp = phase1_stack.enter_context(
                    tc.tile_pool(name="pq", bufs=2, space="PSUM")
                )
                pbonesp = phase1_stack.enter_context(
                    tc.tile_pool(name="pbones", bufs=2, space="PSUM")
                )
                ptp = phase1_stack.enter_context(
                    tc.tile_pool(name="ptp", bufs=1, space="PSUM")
                )

                # q (f 0-7) and k (f 8-15) feature tiles
                for f in range(16):
                    wt = wqks.tile([P, 8, P], BF16, tag="wt")
                    nc.sync.dma_start(wt[:], wqk_d[f])
                    pq = pqp.tile([P, S], F32, tag="pq")
                    for c in range(8):
                        for ch in range(2):
                            sl = slice(ch * 512, (ch + 1) * 512)
                            nc.tensor.matmul(
                                pq[:, sl],
                                wt[:, c, :],
                                xT[:, c, sl],
                                start=(c == 0),
                                stop=(c == 7),
                            )
                    dst = qr if f < 8 else kr
                    t = f % 8
                    # PSUM -> bf16 SBUF (ACT), squares on DVE
                    pqc = pqcp.tile([P, S], BF16, tag="pqc")
                    nc.scalar.activation(pqc[:], pq[:], AF.Copy)
                    sq = sqp.tile([P, S], BF16, tag="sq")
                    nc.gpsimd.tensor_mul(sq[:], pqc[:], pqc[:])
                    # per-token sum of squares over D (pre-rotary; rotary is
                    # norm-preserving per token) -> sqrt rows
                    s2 = s2p.tile([2, S], F32, tag="s2")
                    for ch in range(2):
                        sl = slice(ch * 512, (ch + 1) * 512)
                        pb = pbonesp.tile([2, 512], F32, tag="pb")
                        nc.tensor.matmul(pb[:], bones[:], sq[:, sl])
                        if f < 8:
                            nc.scalar.activation(
                                s2[:, sl], pb[:], AF.Sqrt, bias=eps2q[:, 0:1],
                                scale=1.0 / 64,
                            )
                        else:
                            nc.scalar.activation(
                                s2[:, sl], pb[:], AF.Sqrt, bias=eps2k[:, 0:1],
                                scale=1.0,
                            )
                    dstsrt = srtq if f < 8 else srtk
                    nc.sync.dma_start(dstsrt[2 * t : 2 * t + 2, :], s2[:])
                    # rotary (half-split, transposed layout), all-bf16 on DVE.
                    # sinp rows carry the partition-shifted sin values so both
                    # DVE inputs share a base partition (SB+SB constraint);
                    # only the *output* is partition-shifted.
                    tmp = tmpp.tile([P, S], BF16, tag="tmp")
                    nc.vector.tensor_mul(dst[:, t, :], pqc[:], cosf[:])
                    for hl in range(2):
                        b0 = hl * 64
                        nc.vector.tensor_mul(
                            tmp[b0 : b0 + 32, :],
                            pqc[b0 + 32 : b0 + 64, :],
                            sinp[b0 + 32 : b0 + 64, :],
                        )
                        nc.vector.tensor_mul(
                            tmp[b0 + 32 : b0 + 64, :],
                            pqc[b0 : b0 + 32, :],
                            sinp[b0 : b0 + 32, :],
                        )
                    nc.vector.tensor_add(dst[:, t, :], dst[:, t, :], tmp[:])


                # v in natural [token, H*D] layout, straight into v_aug
                for tt in range(8):
                    pv = pqp.tile([P, S], F32, tag="pq")
                    for c in range(8):
                        for ch in range(2):
                            sl = slice(ch * 512, (ch + 1) * 512)
                            nc.tensor.matmul(
                                pv[:, sl],
                                xT[:, c, tt * P : (tt + 1) * P],
                                wvT[:, c, sl],
                                start=(c == 0),
                                stop=(c == 7),
                            )
                    if K_V3D:
                        for ch in range(2):
                            dst_ap = vaug[:, tt, :].rearrange(
                                "p (h e) -> p h e", h=H
                            )[:, 8 * ch : 8 * ch + 8, 0:64]
                            nc.scalar.activation(
                                dst_ap, pv[:, ch * 512 : (ch + 1) * 512], AF.Copy
                            )
                    else:
                        for h2 in range(H):
                            nc.scalar.activation(
                                vaug[:, tt, h2 * 65 : h2 * 65 + 64],
                                pv[:, h2 * 64 : (h2 + 1) * 64],
                                AF.Copy,
                            )

                # reciprocal of sqrt rows (fast approx, ~18 bits)
                if K_RECF:
                    nc.vector.reciprocal_approx_fast(rinvq[:], srtq[:])
                    nc.vector.reciprocal_approx_fast(rinvk[:], srtk[:])
                else:
                    nc.vector.reciprocal(rinvq[:], srtq[:])
                    nc.vector.reciprocal(rinvk[:], srtk[:])
                # q-side scales: bf16 copy -> dram scratch -> broadcast
                nc.scalar.activation(rq16[:], rinvq[:], AF.Copy)
                nc.sync.dma_start(rq_scr[:, :], rq16[:])
                if K_EXPSC:
                    # k scales transposed to [kpos, h] for Exp's scale operand
                    if K_TRS:
                        for kt in range(8):
                            pt = ptp.tile([P, H], F32, tag="pt")
                            nc.tensor.transpose(
                                pt[:], rinvk[:, kt * P : (kt + 1) * P], identq[0:16, 0:16]
                            )
                            nc.scalar.activation(rkT[:, kt, :], pt[:], AF.Copy)
                    else:
                        rk32 = s2p.tile([32, S], F32, tag="rk32")
                        nc.vector.tensor_copy(rk32[0:16, :], rinvk[:])
                        nc.vector.memset(rk32[16:32, :], 0.0)
                        for kt in range(8):
                            pt = ptp.tile([P, 32], F32, tag="pt32")
                            nc.tensor.transpose(
                                pt[:], rk32[:, kt * P : (kt + 1) * P],
                                identq[:],
                            )
                            nc.scalar.activation(rkT[:, kt, :], pt[:, 0:16], AF.Copy)
                else:
                    # fallback: apply rk to kr via broadcast + DVE mul
                    nc.scalar.activation(rk16[:], rinvk[:], AF.Copy)
                    nc.sync.dma_start(rk_scr[:, :], rk16[:])
                    for t in range(8):
                        bc = bcp.tile([P, S], BF16, tag="bc")
                        for hl in range(2):
                            ro = 2 * t + hl
                            nc.sync.dma_start(
                                bc[hl * 64 : (hl + 1) * 64, :],
                                rk_scr[ro : ro + 1, :].broadcast_to([64, S]),
                            )
                        nc.vector.tensor_mul(kr[:, t, :], kr[:, t, :], bc[:])
                # apply rq to q tiles
                for t in range(8):
                    bc = bcp.tile([P, S], BF16, tag="bc")
                    for hl in range(2):
                        ro = 2 * t + hl
                        nc.sync.dma_start(
                            bc[hl * 64 : (hl + 1) * 64, :],
                            rq_scr[ro : ro + 1, :].broadcast_to([64, S]),
                        )
                    nc.vector.tensor_mul(qr[:, t, :], qr[:, t, :], bc[:])

                phase1_stack.close()

            # ---------------- phase 2: attention ----------------
            with (
                tc.tile_pool(name="expp", bufs=3) as expp,
                tc.tile_pool(name="bc2", bufs=2) as bc2p,
                tc.tile_pool(name="s1p", bufs=2) as s1p,
                tc.tile_pool(name="wop", bufs=8) as wop,
            ):
                phase2_stack = ExitStack()
                psp = phase2_stack.enter_context(
                    tc.tile_pool(name="ps", bufs=2, space="PSUM")
                )
                pop = phase2_stack.enter_context(
                    tc.tile_pool(name="po", bufs=2, space="PSUM")
                )
                # prefetch all Wo weight tiles during attention
                wo_tiles = []
                for o in range(8):
                    wt = wop.tile([P, 8, P], BF16, tag="wo")
                    nc.sync.dma_start(wt[:], wo_d[o])
                    wo_tiles.append(wt)
                # ln(gate), consumed by the ln/exp reciprocal at the tail
                nc.scalar.activation(lngate[:], gate_sb[:], AF.Ln)
                for h in range(H):
                    ft, r0 = h // 2, (h % 2) * 64
                    po = pop.tile([65, S], F32, tag="po")
                    for kt in range(8):
                        q0 = kt * P
                        nsp = S - q0
                        et = expp.tile([P, S], BF16, tag="et")
                        ps = psp.tile([P, S], F32, tag="ps")
                        ofs = 0
                        while ofs < nsp:
                            n = min(512, nsp - ofs)
                            nc.tensor.matmul(
                                ps[:, ofs : ofs + n],
                                kr[r0 : r0 + 64, ft, q0 : q0 + P],
                                qr[r0 : r0 + 64, ft, q0 + ofs : q0 + ofs + n],
                            )
                            ofs += n
                        # exp with k-side rms scale folded in (per-partition)
                        if K_EXPSC:
                            nc.scalar.activation(
                                et[:, 0:nsp], ps[:, 0:nsp], AF.Exp,
                                scale=rkT[:, kt, h : h + 1],
                            )
                        else:
                            nc.scalar.activation(
                                et[:, 0:nsp], ps[:, 0:nsp], AF.Exp
                            )
                        # causal mask on the diagonal tile (GpSimd: idle
                        # engine, keeps the exp->PV chain off the DVE queue)
                        nc.gpsimd.tensor_mul(et[:, 0:P], et[:, 0:P], maskt[:])
                        ofs = 0
                        while ofs < nsp:
                            a = q0 + ofs
                            n = min(512 - (a % 512), nsp - ofs)
                            nc.tensor.matmul(
                                po[:, a : a + n],
                                vaug[:, kt, h * 65 : (h + 1) * 65],
                                et[:, ofs : ofs + n],
                                start=(kt == 0),
                                stop=(kt == 4 * (a // 512) + 3),
                            )
                            ofs += n
                    # denominator row -> base-0 staging -> sums[h] via DMA;
                    # attention rows -> aos. Both on DVE (GpSimd/DMA cannot
                    # read PSUM; ACT is kept free for the exp stream).
                    s1 = s1p.tile([1, S], F32, tag="sd")
                    if K_GPS:
                        nc.vector.tensor_copy(s1[:], po[64:65, :])
                    else:
                        nc.scalar.activation(s1[:], po[64:65, :], AF.Copy)
                    nc.sync.dma_start(sums[h : h + 1, :], s1[:])
                    nc.vector.tensor_copy(aos[r0 : r0 + 64, ft, :], po[0:64, :])

                # scale = gate/denominator = exp(ln(gate) - ln(den)):
                # ln+exp live in one ACT table with the softmax exp, so this
                # replaces the 7.8us multi-pass DVE reciprocal on the tail.
                nc.scalar.activation(rd[:], sums[:], AF.Ln)
                nc.vector.tensor_sub(rd[:], lngate[:], rd[:])
                nc.scalar.activation(sc16[:], rd[:], AF.Exp)
                nc.sync.dma_start(sc_scr[:, :], sc16[:])
                for ct in range(8):
                    bc = bc2p.tile([P, S], BF16, tag="bc2")
                    for hl in range(2):
                        ro = 2 * ct + hl
                        nc.sync.dma_start(
                            bc[hl * 64 : (hl + 1) * 64, :],
                            sc_scr[ro : ro + 1, :].broadcast_to([64, S]),
                        )
                    nc.vector.tensor_mul(aos[:, ct, :], aos[:, ct, :], bc[:])

                phase2_stack.close()
                # ---------- phase 3: output projection ----------
                with (
                    tc.tile_pool(name="osb", bufs=2) as osbp,
                    tc.tile_pool(name="pw", bufs=2, space="PSUM") as pwp,
                ):
                    for o in range(8):
                        wt = wo_tiles[o]
                        pw = pwp.tile([P, S], F32, tag="pw")
                        for c in range(8):
                            for ch in range(2):
                                sl = slice(ch * 512, (ch + 1) * 512)
                                nc.tensor.matmul(
                                    pw[:, sl],
                                    wt[:, c, :],
                                    aos[:, c, sl],
                                    start=(c == 0),
                                    stop=(c == 7),
                                )
                        ot = osbp.tile([P, S], F32, tag="ot")
                        nc.scalar.activation(ot[:], pw[:], AF.Copy)
                        nc.sync.dma_start(
                            outt_d[o * P : (o + 1) * P, :], ot[:]
                        )
    return nc


def prepare_inputs(x, Wqkv, Wo, gate_w, gate_b, cos_cache, sin_cache, position_ids):
    """Host-side sharding + layout prep. Returns per-core input maps."""
    x = np.asarray(x, dtype=np.float32)
    WqkvT = np.asarray(Wqkv, dtype=np.float32).T  # [C, 3C]
    wqk_r = np.ascontiguousarray(
        WqkvT[:, : 2 * C].reshape(8, P, 16, P).transpose(2, 1, 0, 3)
    ).astype(BF16NP)  # [f, p, c, d] for q,k
    wvt_r = np.ascontiguousarray(
        WqkvT[:, 2 * C :].reshape(8, P, C)
    ).astype(BF16NP)  # [c, p, vfeat]
    WoT = np.asarray(Wo, dtype=np.float32).T  # [C, C]
    wo_r = np.ascontiguousarray(
        WoT.reshape(8, P, 8, P).transpose(2, 1, 0, 3)
    ).astype(BF16NP)
    gwT = np.asarray(gate_w, dtype=np.float32).T  # [C, H]
    gw_r = np.ascontiguousarray(
        gwT.reshape(8, P, H).transpose(1, 0, 2).reshape(P, P)
    ).astype(BF16NP)
    gb_r = np.asarray(gate_b, dtype=np.float32).reshape(H, 1)
    maskt = np.triu(np.ones((P, P), dtype=np.float32)).astype(BF16NP)
    bones = np.zeros((P, 2), dtype=np.float32)
    bones[0:64, 0] = 1.0
    bones[64:128, 1] = 1.0
    bones = bones.astype(BF16NP)
    identq = np.eye(32, dtype=np.float32)
    cos_cache = np.asarray(cos_cache, dtype=np.float32)
    sin_cache = np.asarray(sin_cache, dtype=np.float32)
    position_ids = np.asarray(position_ids)

    in_maps = []
    for b in range(NCORES):
        xs = x[b * S : (b + 1) * S, :]
        pos = position_ids[b * S : (b + 1) * S]
        ct = cos_cache[pos].T  # [32, S]
        st = sin_cache[pos].T
        cosf = np.ascontiguousarray(np.tile(ct, (4, 1))).astype(BF16NP)
        # rows 0-31: -st (consumed by the shifted-output mul writing rows
        # 32-63), rows 32-63: st (writing rows 0-31); tiled for both halves.
        sinp = np.ascontiguousarray(
            np.tile(np.concatenate([-st, st], axis=0), (2, 1))
        ).astype(BF16NP)
        in_maps.append(
            {
                "xt": np.ascontiguousarray(xs.T).astype(BF16NP),
                "wqk": wqk_r,
                "wvt": wvt_r,
                "wo": wo_r,
                "gw": gw_r,
                "gb": gb_r,
                "cosf": cosf,
                "sinp": sinp,
                "maskt": maskt,
                "bones": bones,
                "identq": identq,
            }
        )
    return in_maps


_CACHED_NC = None


def kernel(
    x,
    Wqkv,
    Wo,
    gate_w,
    gate_b,
    cos_cache,
    sin_cache,
    cu_seqlens,
    position_ids,
    max_seqlen,
):
    global _CACHED_NC
    in_maps = prepare_inputs(
        x, Wqkv, Wo, gate_w, gate_b, cos_cache, sin_cache, position_ids
    )
    if _CACHED_NC is None:
        _CACHED_NC = build_program()
    res = bass_utils.run_bass_kernel_spmd(
        _CACHED_NC, in_maps, core_ids=list(range(NCORES))
    )
    out = np.empty((NCORES * S, C), dtype=np.float32)
    for b in range(NCORES):
        out[b * S : (b + 1) * S, :] = res.results[b]["outt"].T
    return out


# revision 22
# speedup vs baseline: 1.1604x; 1.1604x over previous
"""Causal varlen self-attention (qk-norm + rotary + head gating) on 8 trn2 cores.

Sharding: data-parallel by sequence — 8 packed equal-length sequences, one per
NeuronCore; weights replicated. No collectives.

v2: all matmuls in bf16 (1 cycle/row on the PE vs ~2.3 for fp32-HIGH mode),
elementwise work in bf16 (DVE 2x mode), V projected directly into natural
[token, head*D] layout (no PE transposes / PSUM evacuation chain), k-side
rms scale folded into the Exp activation's per-partition scale operand,
reciprocal_approx_fast instead of multi-pass InstReciprocal, and PSUM
evacuation moved to the otherwise-idle GpSimd engine.

Per-core dataflow (S=1024 tokens, C=1024 hidden, H=16 heads, D=64):
  phase 1: qkv^T = W-tiles x x^T (bf16). rotary applied on DVE in bf16;
           per-token sumsq via ones-block matmul of square(pq) (rotation
           preserves per-token norms, so sumsq is taken pre-rotary).
           q gets 1/sqrt(ms/64+eps) applied via broadcast-DMA + DVE mul;
           k's 1/sqrt(ms+64 eps) (D^-.5 folded) is transposed into [kpos, h]
           columns and later fed to Exp as its per-partition scale.
           v computed in natural [tok, H*D] layout: x^T-tile-stationary x
           Wv^T-moving, PSUM evacuated straight into v_aug (ones column
           appended so the softmax denominator falls out of the PV matmul).
  phase 2: per (head, k-tile): scores = k-tile-stationary x q-moving,
           et = Exp(scores * rk) on ACT, causal mask on the diagonal tile,
           PV accumulates [65, S]. GpSimd evacuates attention rows to aos
           and denominator rows to staging; gate*1/denominator applied as
           one broadcast multiply.
  phase 3: out^T = Wo^T-tiles-stationary x aos-moving.
"""

import os
import sys
from contextlib import ExitStack

sys.path.insert(0, "/opt/trn_rl_repo")

K_GPS = os.environ.get("K_GPS", "1") == "1"  # gpsimd PSUM evacuation
K_EXPSC = os.environ.get("K_EXPSC", "1") == "1"  # exp scale-AP rk fold
K_V3D = os.environ.get("K_V3D", "1") == "1"  # 3D strided ACT out for v evac
K_TRS = os.environ.get("K_TRS", "1") == "1"  # 16-partition transpose for rkT
# reciprocal_approx_fast (custom DVE) fails this walrus build's codegen
# ("ISA wrong length") — default to the stock multi-pass InstReciprocal.
K_RECF = os.environ.get("K_RECF", "0") == "1"

import numpy as np
import ml_dtypes
import bass_rust
import concourse.bass as bass
import concourse.tile as tile
from concourse import mybir
from concourse import bass_utils

P = 128
S = 1024  # tokens per sequence (= per core)
C = 1024  # hidden
H = 16
D = 64
NCORES = 8
F32 = mybir.dt.float32
BF16 = mybir.dt.bfloat16
AF = mybir.ActivationFunctionType
BF16NP = ml_dtypes.bfloat16


class TC(tile.TileContext):
    """TileContext that rewrites every instruction to carry at most ONE sem wait.

    This container's walrus rejects instructions with more than one sync wait
    command (matmul LDW structs, CTRL drains, ...). Tile's wait-assignment
    pass attaches one wait per producer proc, so fan-in instructions get
    several. After scheduling, hoist all but the last wait of each
    instruction onto same-engine NOPs inserted immediately before it —
    identical synchronization semantics, one wait per encoded instruction.
    """

    _split_seq = 0
    split_waits = True

    def schedule_and_allocate(self, *args, **kwargs):
        ret = super().schedule_and_allocate(*args, **kwargs)
        if not self.split_waits:
            return ret
        nc = self.nc
        for fn in nc.m.functions:
            for blk in fn.blocks:
                insts = blk.instructions
                out = []
                changed = False
                for ins in insts:
                    si = getattr(ins, "sync_info", None)
                    waits = list(si.on_wait) if si is not None else []
                    if len(waits) > 1:
                        changed = True
                        for w in waits[:-1]:
                            TC._split_seq += 1
                            nop = bass_rust.InstNoOp(
                                name=f"I-splitw-{TC._split_seq}",
                                engine=ins.engine,
                                ins=[],
                                outs=[],
                            )
                            nop.sync_info = bass_rust.SyncInfo(
                                on_wait=[w], on_update=[]
                            )
                            out.append(nop)
                        ins.sync_info = bass_rust.SyncInfo(
                            on_wait=[waits[-1]], on_update=list(si.on_update)
                        )
                    out.append(ins)
                if changed:
                    blk.instructions = out
        return ret


def build_program(split_waits=True):
    nc = bass.Bass("TRN2", target_bir_lowering=False, debug=False)
    dt = nc.dram_tensor
    xt_d = dt("xt", [C, S], BF16, kind="ExternalInput").ap()
    wqk_d = dt("wqk", [16, P, 8, P], BF16, kind="ExternalInput").ap()
    wvt_d = dt("wvt", [8, P, C], BF16, kind="ExternalInput").ap()
    wo_d = dt("wo", [8, P, 8, P], BF16, kind="ExternalInput").ap()
    gw_d = dt("gw", [P, P], BF16, kind="ExternalInput").ap()
    gb_d = dt("gb", [H, 1], F32, kind="ExternalInput").ap()
    cosf_d = dt("cosf", [P, S], BF16, kind="ExternalInput").ap()
    sinp_d = dt("sinp", [P, S], BF16, kind="ExternalInput").ap()
    maskt_d = dt("maskt", [P, P], BF16, kind="ExternalInput").ap()
    bones_d = dt("bones", [P, 2], BF16, kind="ExternalInput").ap()
    identq_d = dt("identq", [32, 32], F32, kind="ExternalInput").ap()
    outt_d = dt("outt", [C, S], F32, kind="ExternalOutput").ap()
    rq_scr = dt("rq_scr", [H, S], BF16).ap()
    rk_scr = dt("rk_scr", [H, S], BF16).ap()
    sc_scr = dt("sc_scr", [H, S], BF16).ap()

    with TC(nc) as tc:
        tc.split_waits = split_waits
        with (
            tc.tile_pool(name="const", bufs=1) as constp,
            tc.tile_pool(name="resid", bufs=1) as resid,
            tc.tile_pool(name="stats", bufs=1) as stats,
        ):
            cosf = constp.tile([P, S], BF16, tag="cosf")
            sinp = constp.tile([P, S], BF16, tag="sinp")
            maskt = constp.tile([P, P], BF16, tag="maskt")
            bones = constp.tile([P, 2], BF16, tag="bones")
            identq = constp.tile([32, 32], F32, tag="identq")
            gw_sb = constp.tile([P, P], BF16, tag="gw")
            gb_sb = constp.tile([H, 1], F32, tag="gb")
            nc.sync.dma_start(cosf[:], cosf_d[:])
            nc.sync.dma_start(sinp[:], sinp_d[:])
            nc.sync.dma_start(maskt[:], maskt_d[:])
            nc.sync.dma_start(bones[:], bones_d[:])
            nc.sync.dma_start(identq[:], identq_d[:])
            nc.sync.dma_start(gw_sb[:], gw_d[:])
            nc.sync.dma_start(gb_sb[:], gb_d[:])

            qr = resid.tile([P, 8, S], BF16, tag="qr")
            kr = resid.tile([P, 8, S], BF16, tag="kr")
            vaug = resid.tile([P, 8, H * 65], BF16, tag="vaug")
            aos = resid.tile([P, 8, S], BF16, tag="aos")

            gate_sb = stats.tile([H, S], F32, tag="gate")
            srtq = stats.tile([H, S], F32, tag="srtq")
            srtk = stats.tile([H, S], F32, tag="srtk")
            rinvq = stats.tile([H, S], F32, tag="rinvq")
            rinvk = stats.tile([H, S], F32, tag="rinvk")
            rq16 = stats.tile([H, S], BF16, tag="rq16")
            rk16 = stats.tile([H, S], BF16, tag="rk16")
            rkT = stats.tile([P, 8, H], F32, tag="rkT")
            sums = stats.tile([H, S], F32, tag="sums")
            lngate = stats.tile([H, S], F32, tag="lngate")
            dn4 = stats.tile([P, 4 * S], F32, tag="dn4")
            rd = stats.tile([H, S], F32, tag="rd")
            sc16 = stats.tile([H, S], BF16, tag="sc16")
            eps2q = stats.tile([2, 1], F32, tag="eps2q")
            eps2k = stats.tile([2, 1], F32, tag="eps2k")
            nc.vector.memset(eps2q[:], 1e-6)
            nc.vector.memset(eps2k[:], 6.4e-5)

            # ones columns of v_aug (col 64 of each head's 65-wide block)
            for kt in range(8):
                ones_ap = vaug[:, kt, :].rearrange("p (h e) -> p h e", h=H)[
                    :, :, 64:65
                ]
                nc.vector.memset(ones_ap, 1.0)

            # ---------------- phase 1: projections ----------------
            with (
                tc.tile_pool(name="xp", bufs=1) as xp,
                tc.tile_pool(name="wvp", bufs=1) as wvp,
                tc.tile_pool(name="wqks", bufs=2) as wqks,
                tc.tile_pool(name="pqc", bufs=2) as pqcp,
                tc.tile_pool(name="sqp", bufs=2) as sqp,
                tc.tile_pool(name="tmp", bufs=2) as tmpp,
                tc.tile_pool(name="s2p", bufs=2) as s2p,
                tc.tile_pool(name="bcp", bufs=1) as bcp,
            ):
                xT = xp.tile([P, 8, S], BF16, tag="xT")
                for c in range(8):
                    for ch in range(2):
                        sl = slice(ch * 512, (ch + 1) * 512)
                        nc.sync.dma_start(
                            xT[:, c, sl], xt_d[c * P : (c + 1) * P, sl]
                        )
                wvT = wvp.tile([P, 8, C], BF16, tag="wvT")
                for c in range(8):
                    nc.sync.dma_start(wvT[:, c, :], wvt_d[c])

                # gate logits, one 512-chunk at a time (Sigmoid table first)
                with tc.tile_pool(name="pgate", bufs=2, space="PSUM") as pgatep:
                    for ch in range(2):
                        sl = slice(ch * 512, (ch + 1) * 512)
                        pgate = pgatep.tile([H, 512], F32, tag="pgate")
                        for c in range(8):
                            nc.tensor.matmul(
                                pgate[:],
                                gw_sb[:, c * H : (c + 1) * H],
                                xT[:, c, sl],
                                start=(c == 0),
                                stop=(c == 7),
                            )
                        nc.scalar.activation(
                            gate_sb[:, sl], pgate[:], AF.Sigmoid, bias=gb_sb[:, 0:1]
                        )

                phase1_stack = ExitStack()
                pqp = phase1_stack.enter_context(
                    tc.tile_pool(name="pq", bufs=2, space="PSUM")
                )
                pbonesp = phase1_stack.enter_context(
                    tc.tile_pool(name="pbones", bufs=2, space="PSUM")
                )
                ptp = phase1_stack.enter_context(
                    tc.tile_pool(name="ptp", bufs=1, space="PSUM")
                )

                # q (f 0-7) and k (f 8-15) feature tiles
                for f in range(16):
                    wt = wqks.tile([P, 8, P], BF16, tag="wt")
                    nc.sync.dma_start(wt[:], wqk_d[f])
                    pq = pqp.tile([P, S], F32, tag="pq")
                    for c in range(8):
                        for ch in range(2):
                            sl = slice(ch * 512, (ch + 1) * 512)
                            nc.tensor.matmul(
                                pq[:, sl],
                                wt[:, c, :],
                                xT[:, c, sl],
                                start=(c == 0),
                                stop=(c == 7),
                            )
                    dst = qr if f < 8 else kr
                    t = f % 8
                    # PSUM -> bf16 SBUF (ACT), squares on DVE
                    pqc = pqcp.tile([P, S], BF16, tag="pqc")
                    nc.scalar.activation(pqc[:], pq[:], AF.Copy)
                    sq = sqp.tile([P, S], BF16, tag="sq")
                    nc.vector.tensor_mul(sq[:], pqc[:], pqc[:])
                    # per-token sum of squares over D (pre-rotary; rotary is
                    # norm-preserving per token) -> sqrt rows
                    s2 = s2p.tile([2, S], F32, tag="s2")
                    for ch in range(2):
                        sl = slice(ch * 512, (ch + 1) * 512)
                        pb = pbonesp.tile([2, 512], F32, tag="pb")
                        nc.tensor.matmul(pb[:], bones[:], sq[:, sl])
                        if f < 8:
                            nc.scalar.activation(
                                s2[:, sl], pb[:], AF.Sqrt, bias=eps2q[:, 0:1],
                                scale=1.0 / 64,
                            )
                        else:
                            nc.scalar.activation(
                                s2[:, sl], pb[:], AF.Sqrt, bias=eps2k[:, 0:1],
                                scale=1.0,
                            )
                    dstsrt = srtq if f < 8 else srtk
                    nc.sync.dma_start(dstsrt[2 * t : 2 * t + 2, :], s2[:])
                    # rotary (half-split, transposed layout), all-bf16 on DVE.
                    # sinp rows carry the partition-shifted sin values so both
                    # DVE inputs share a base partition (SB+SB constraint);
                    # only the *output* is partition-shifted.
                    tmp = tmpp.tile([P, S], BF16, tag="tmp")
                    nc.vector.tensor_mul(dst[:, t, :], pqc[:], cosf[:])
                    for hl in range(2):
                        b0 = hl * 64
                        nc.vector.tensor_mul(
                            tmp[b0 : b0 + 32, :],
                            pqc[b0 + 32 : b0 + 64, :],
                            sinp[b0 + 32 : b0 + 64, :],
                        )
                        nc.vector.tensor_mul(
                            tmp[b0 + 32 : b0 + 64, :],
                            pqc[b0 : b0 + 32, :],
                            sinp[b0 : b0 + 32, :],
                        )
                    nc.vector.tensor_add(dst[:, t, :], dst[:, t, :], tmp[:])


                # v in natural [token, H*D] layout, straight into v_aug
                for tt in range(8):
                    pv = pqp.tile([P, S], F32, tag="pq")
                    for c in range(8):
                        for ch in range(2):
                            sl = slice(ch * 512, (ch + 1) * 512)
                            nc.tensor.matmul(
                                pv[:, sl],
                                xT[:, c, tt * P : (tt + 1) * P],
                                wvT[:, c, sl],
                                start=(c == 0),
                                stop=(c == 7),
                            )
                    if K_V3D:
                        for ch in range(2):
                            dst_ap = vaug[:, tt, :].rearrange(
                                "p (h e) -> p h e", h=H
                            )[:, 8 * ch : 8 * ch + 8, 0:64]
                            nc.scalar.activation(
                                dst_ap, pv[:, ch * 512 : (ch + 1) * 512], AF.Copy
                            )
                    else:
                        for h2 in range(H):
                            nc.scalar.activation(
                                vaug[:, tt, h2 * 65 : h2 * 65 + 64],
                                pv[:, h2 * 64 : (h2 + 1) * 64],
                                AF.Copy,
                            )

                # reciprocal of sqrt rows (fast approx, ~18 bits)
                if K_RECF:
                    nc.vector.reciprocal_approx_fast(rinvq[:], srtq[:])
                    nc.vector.reciprocal_approx_fast(rinvk[:], srtk[:])
                else:
                    nc.vector.reciprocal(rinvq[:], srtq[:])
                    nc.vector.reciprocal(rinvk[:], srtk[:])
                # q-side scales: bf16 copy -> dram scratch -> broadcast
                nc.scalar.activation(rq16[:], rinvq[:], AF.Copy)
                nc.sync.dma_start(rq_scr[:, :], rq16[:])
                if K_EXPSC:
                    # k scales transposed to [kpos, h] for Exp's scale operand
                    if K_TRS:
                        for kt in range(8):
                            pt = ptp.tile([P, H], F32, tag="pt")
                            nc.tensor.transpose(
                                pt[:], rinvk[:, kt * P : (kt + 1) * P], identq[0:16, 0:16]
                            )
                            nc.scalar.activation(rkT[:, kt, :], pt[:], AF.Copy)
                    else:
                        rk32 = s2p.tile([32, S], F32, tag="rk32")
                        nc.vector.tensor_copy(rk32[0:16, :], rinvk[:])
                        nc.vector.memset(rk32[16:32, :], 0.0)
                        for kt in range(8):
                            pt = ptp.tile([P, 32], F32, tag="pt32")
                            nc.tensor.transpose(
                                pt[:], rk32[:, kt * P : (kt + 1) * P],
                                identq[:],
                            )
                            nc.scalar.activation(rkT[:, kt, :], pt[:, 0:16], AF.Copy)
                else:
                    # fallback: apply rk to kr via broadcast + DVE mul
                    nc.scalar.activation(rk16[:], rinvk[:], AF.Copy)
                    nc.sync.dma_start(rk_scr[:, :], rk16[:])
                    for t in range(8):
                        bc = bcp.tile([P, S], BF16, tag="bc")
                        for hl in range(2):
                            ro = 2 * t + hl
                            nc.sync.dma_start(
                                bc[hl * 64 : (hl + 1) * 64, :],
                                rk_scr[ro : ro + 1, :].broadcast_to([64, S]),
                            )
                        nc.vector.tensor_mul(kr[:, t, :], kr[:, t, :], bc[:])
                # apply rq to q tiles; one broadcast DMA per half covers
                # all 8 tiles (Sync dispatch is ~600ns per DMA)
                bc8 = bcp.tile([P, 8, S], BF16, tag="bc8")
                for hl in range(2):
                    nc.sync.dma_start(
                        bc8[hl * 64 : (hl + 1) * 64, :, :],
                        rq_scr[hl::2, :]
                        .rearrange("(o r) s -> o r s", o=1)
                        .broadcast_to([64, 8, S]),
                    )
                for t in range(8):
                    nc.vector.tensor_mul(
                        qr[:, t, :], qr[:, t, :], bc8[:, t, :]
                    )

                phase1_stack.close()

            # ---------------- phase 2: attention ----------------
            with (
                tc.tile_pool(name="expp", bufs=3) as expp,
                tc.tile_pool(name="bc2", bufs=1) as bc2p,
                tc.tile_pool(name="s1p", bufs=2) as s1p,
                tc.tile_pool(name="wop", bufs=8) as wop,
            ):
                phase2_stack = ExitStack()
                psp = phase2_stack.enter_context(
                    tc.tile_pool(name="ps", bufs=2, space="PSUM")
                )
                pop = phase2_stack.enter_context(
                    tc.tile_pool(name="po", bufs=2, space="PSUM")
                )
                # prefetch all Wo weight tiles during attention
                wo_tiles = []
                for o in range(8):
                    wt = wop.tile([P, 8, P], BF16, tag="wo")
                    nc.sync.dma_start(wt[:], wo_d[o])
                    wo_tiles.append(wt)
                # ln(gate), consumed by the ln/exp reciprocal at the tail
                nc.scalar.activation(lngate[:], gate_sb[:], AF.Ln)
                for h in range(H):
                    ft, r0 = h // 2, (h % 2) * 64
                    po = pop.tile([65, S], F32, tag="po")
                    for kt in range(8):
                        q0 = kt * P
                        nsp = S - q0
                        et = expp.tile([P, S], BF16, tag="et")
                        ps = psp.tile([P, S], F32, tag="ps")
                        ofs = 0
                        while ofs < nsp:
                            n = min(512, nsp - ofs)
                            nc.tensor.matmul(
                                ps[:, ofs : ofs + n],
                                kr[r0 : r0 + 64, ft, q0 : q0 + P],
                                qr[r0 : r0 + 64, ft, q0 + ofs : q0 + ofs + n],
                            )
                            ofs += n
                        # exp with k-side rms scale folded in (per-partition)
                        if K_EXPSC:
                            nc.scalar.activation(
                                et[:, 0:nsp], ps[:, 0:nsp], AF.Exp,
                                scale=rkT[:, kt, h : h + 1],
                            )
                        else:
                            nc.scalar.activation(
                                et[:, 0:nsp], ps[:, 0:nsp], AF.Exp
                            )
                        # causal mask on the diagonal tile
                        nc.vector.tensor_mul(et[:, 0:P], et[:, 0:P], maskt[:])
                        ofs = 0
                        while ofs < nsp:
                            a = q0 + ofs
                            n = min(512 - (a % 512), nsp - ofs)
                            nc.tensor.matmul(
                                po[:, a : a + n],
                                vaug[:, kt, h * 65 : (h + 1) * 65],
                                et[:, ofs : ofs + n],
                                start=(kt == 0),
                                stop=(kt == 4 * (a // 512) + 3),
                            )
                            ofs += n
                    # denominator row -> dn4 staging at partition 32*(h//4)
                    # (4 batched DMAs to sums later); attention rows -> aos.
                    # Both on DVE (GpSimd/DMA cannot read PSUM).
                    pi, bi = h // 4, h % 4
                    nc.vector.tensor_copy(
                        dn4[32 * pi : 32 * pi + 1, bi * S : (bi + 1) * S],
                        po[64:65, :],
                    )
                    nc.vector.tensor_copy(aos[r0 : r0 + 64, ft, :], po[0:64, :])

                # scale = gate/denominator = exp(ln(gate) - ln(den)):
                # ln+exp live in one ACT table with the softmax exp, so this
                # replaces the 7.8us multi-pass DVE reciprocal on the tail.
                for pi in range(4):
                    nc.sync.dma_start(
                        sums[4 * pi : 4 * pi + 4, :],
                        dn4[32 * pi : 32 * pi + 1, :],
                    )
                nc.scalar.activation(rd[:], sums[:], AF.Ln)
                nc.vector.tensor_sub(rd[:], lngate[:], rd[:])
                nc.scalar.activation(sc16[:], rd[:], AF.Exp)
                nc.sync.dma_start(sc_scr[:, :], sc16[:])
                bs8 = bc2p.tile([P, 8, S], BF16, tag="bs8")
                for hl in range(2):
                    nc.sync.dma_start(
                        bs8[hl * 64 : (hl + 1) * 64, :, :],
                        sc_scr[hl::2, :]
                        .rearrange("(o r) s -> o r s", o=1)
                        .broadcast_to([64, 8, S]),
                    )
                for ct in range(8):
                    nc.vector.tensor_mul(
                        aos[:, ct, :], aos[:, ct, :], bs8[:, ct, :]
                    )

                phase2_stack.close()
                # ---------- phase 3: output projection ----------
                with (
                    tc.tile_pool(name="osb", bufs=2) as osbp,
                    tc.tile_pool(name="pw", bufs=2, space="PSUM") as pwp,
                ):
                    for o in range(8):
                        wt = wo_tiles[o]
                        pw = pwp.tile([P, S], F32, tag="pw")
                        for c in range(8):
                            for ch in range(2):
                                sl = slice(ch * 512, (ch + 1) * 512)
                                nc.tensor.matmul(
                                    pw[:, sl],
                                    wt[:, c, :],
                                    aos[:, c, sl],
                                    start=(c == 0),
                                    stop=(c == 7),
                                )
                        ot = osbp.tile([P, S], F32, tag="ot")
                        nc.scalar.activation(ot[:], pw[:], AF.Copy)
                        nc.sync.dma_start(
                            outt_d[o * P : (o + 1) * P, :], ot[:]
                        )
    return nc


def prepare_inputs(x, Wqkv, Wo, gate_w, gate_b, cos_cache, sin_cache, position_ids):
    """Host-side sharding + layout prep. Returns per-core input maps."""
    x = np.asarray(x, dtype=np.float32)
    WqkvT = np.asarray(Wqkv, dtype=np.float32).T  # [C, 3C]
    wqk_r = np.ascontiguousarray(
        WqkvT[:, : 2 * C].reshape(8, P, 16, P).transpose(2, 1, 0, 3)
    ).astype(BF16NP)  # [f, p, c, d] for q,k
    wvt_r = np.ascontiguousarray(
        WqkvT[:, 2 * C :].reshape(8, P, C)
    ).astype(BF16NP)  # [c, p, vfeat]
    WoT = np.asarray(Wo, dtype=np.float32).T  # [C, C]
    wo_r = np.ascontiguousarray(
        WoT.reshape(8, P, 8, P).transpose(2, 1, 0, 3)
    ).astype(BF16NP)
    gwT = np.asarray(gate_w, dtype=np.float32).T  # [C, H]
    gw_r = np.ascontiguousarray(
        gwT.reshape(8, P, H).transpose(1, 0, 2).reshape(P, P)
    ).astype(BF16NP)
    gb_r = np.asarray(gate_b, dtype=np.float32).reshape(H, 1)
    maskt = np.triu(np.ones((P, P), dtype=np.float32)).astype(BF16NP)
    bones = np.zeros((P, 2), dtype=np.float32)
    bones[0:64, 0] = 1.0
    bones[64:128, 1] = 1.0
    bones = bones.astype(BF16NP)
    identq = np.eye(32, dtype=np.float32)
    cos_cache = np.asarray(cos_cache, dtype=np.float32)
    sin_cache = np.asarray(sin_cache, dtype=np.float32)
    position_ids = np.asarray(position_ids)

    in_maps = []
    for b in range(NCORES):
        xs = x[b * S : (b + 1) * S, :]
        pos = position_ids[b * S : (b + 1) * S]
        ct = cos_cache[pos].T  # [32, S]
        st = sin_cache[pos].T
        cosf = np.ascontiguousarray(np.tile(ct, (4, 1))).astype(BF16NP)
        # rows 0-31: -st (consumed by the shifted-output mul writing rows
        # 32-63), rows 32-63: st (writing rows 0-31); tiled for both halves.
        sinp = np.ascontiguousarray(
            np.tile(np.concatenate([-st, st], axis=0), (2, 1))
        ).astype(BF16NP)
        in_maps.append(
            {
                "xt": np.ascontiguousarray(xs.T).astype(BF16NP),
                "wqk": wqk_r,
                "wvt": wvt_r,
                "wo": wo_r,
                "gw": gw_r,
                "gb": gb_r,
                "cosf": cosf,
                "sinp": sinp,
                "maskt": maskt,
                "bones": bones,
                "identq": identq,
            }
        )
    return in_maps


_CACHED_NC = None


def kernel(
    x,
    Wqkv,
    Wo,
    gate_w,
    gate_b,
    cos_cache,
    sin_cache,
    cu_seqlens,
    position_ids,
    max_seqlen,
):
    global _CACHED_NC
    in_maps = prepare_inputs(
        x, Wqkv, Wo, gate_w, gate_b, cos_cache, sin_cache, position_ids
    )
    if _CACHED_NC is None:
        _CACHED_NC = build_program()
    res = bass_utils.run_bass_kernel_spmd(
        _CACHED_NC, in_maps, core_ids=list(range(NCORES))
    )
    out = np.empty((NCORES * S, C), dtype=np.float32)
    for b in range(NCORES):
        out[b * S : (b + 1) * S, :] = res.results[b]["outt"].T
    return out
